# revision 23
# baseline (speedup 1.0000x reference)
"""Trainium2 Bass kernel v3 for scatter_memory (nn_Memory_90031104459201).

Math (per reference.py):
    feat_n = l2norm(feat)                         [65536, 256]
    S      = segment_sum(feat_n, label, 1000)     [1000, 256]
    bc     = l2norm(S); w = <mem, bc>
    new_m  = l2norm(w*mem + (1-w)*bc)
    logits = feat_n @ [new_m; src].T              [65536, 2000]
    loss   = mean(logsumexp(logits)) - <S, new_m>_F / 65536

v3 strategy (8 cores, data-parallel rows, 8192 rows/core):
  - HOST SORTS ROWS BY LABEL (loss is row-permutation invariant): each
    core's 8192 rows then cover a ~125-class band.  The one-hot for the
    segment-sum shrinks from [8192,1024] to [8192,CBAND] (CBAND~192),
    i.e. 1.5 MiB instead of 8 MiB of DMA, and the segment-sum matmul
    writes only a [128, CBAND] psum band.
  - Cross-core reduction becomes an AllGather of the 8 per-core bands
    (cost model: 15us constant, vs AllReduce 15us*1.875) + 8 bf16
    adds on DVE to reconstruct the global S.
  - new_memory in closed form with g=1-w (no flags: empty classes give
    w=0 naturally); rsqrt via exp(-0.5*ln(x)) so ACT keeps ONE table
    set; per-class a/b coefs broadcast to 128 partitions via a K=1
    matmul; <S,new_m> via two stt accum_out ops.
  - Logits row-tiles of 128: the 64 source-half tiles [128,1000] are
    fully independent and keep ACT busy from ~5us while the collective
    + chain complete; memory-half tiles run after new_m is ready,
    paired two-at-a-time [128,2048].  All row-sums of exp() are DVE
    tensor_reduce (no ACT accumulator reads).  ACT is the bottleneck
    engine at ~122us busy.
  - CBAND and the 8 band offsets are input-dependent compile constants
    (same for all cores -> single SPMD program); any label distribution
    just changes the constants, degenerating gracefully to CBAND=1000.
"""

import numpy as np
import ml_dtypes

import concourse.bass as bass
import concourse.bass_isa as bass_isa
import concourse.mybir as mybir
import concourse.tile as tile
from concourse import bacc
from concourse.bass_utils import run_bass_kernel_spmd

F32 = mybir.dt.float32
BF16 = mybir.dt.bfloat16
F16 = mybir.dt.float16
FP8 = mybir.dt.float8e4
AF = mybir.ActivationFunctionType
ALU = mybir.AluOpType
DR = mybir.MatmulPerfMode.DoubleRow

N_CORES = 8
N_TOTAL = 65536
R = N_TOTAL // N_CORES   # 8192 rows/core
D = 256
C = 1000
P = 128
TP = 32                  # row-pair tiles (256 rows each)
T = 64                   # logits row tiles of 128
W = 1000                 # class width per half (exact, no padding)
H = T // 2
EPS = 1e-12

# schedule knobs: a2 = source-half logit tiles (ACT filler work).
# Segment boundaries: how many a2 tiles are emitted before each stage
# of the NM chain goes into the (in-order) engine programs.
A2_DUMP = 8              # a2 tiles interleaved before the dump copies
A2_SS = 20               # a2 tiles emitted by the end of the ss phase
A2_LNN = 30              # before the invn ln/exp pair
A2_CH = 34               # before the chain DVE block + ln2/exp2
A2_MO = 36               # before abbc/mo8 writes; rest after
B_SINGLES = 0            # memory-half tiles done singly at the seam

_CACHE = {}


def _patch_act_tables():
    """Map exp/ln to the combined natural_log_exp_and_others set so the
    ACT engine loads its spline tables exactly once."""
    import concourse.bacc as bacc_mod
    if getattr(bacc_mod, "_act_tables_patched", False):
        return
    orig = bacc_mod.get_activation_tables

    def patched(arch):
        tabs = orig(arch)
        combined = "natural_log_exp_and_others"
        if combined in tabs:
            keep = tabs[combined]
            tabs = {k: (v if k == combined else (v - keep))
                    for k, v in tabs.items()}
        return tabs

    bacc_mod.get_activation_tables = patched
    bacc_mod._act_tables_patched = True


def _build(cband, los, debug=False):
    _patch_act_tables()
    nc = bacc.Bacc("TRN2", num_devices=N_CORES)

    fT8_d = nc.dram_tensor("fT8", [P, 2 * R], FP8, kind="ExternalInput")
    fg8_d = nc.dram_tensor("fg8", [P, TP * 2 * D], FP8, kind="ExternalInput")
    ohb_d = nc.dram_tensor("ohb", [P, TP * 2 * cband], FP8, kind="ExternalInput")
    mo8s_d = nc.dram_tensor("mo8s", [P, 2 * W], FP8, kind="ExternalInput")
    memf_d = nc.dram_tensor("memf", [P, 2 * W], BF16, kind="ExternalInput")
    out_d = nc.dram_tensor("out", [1, 2], F32, kind="ExternalOutput")
    dbg = None
    if debug:
        dbg = {
            "dbg_sg": nc.dram_tensor("dbg_sg", [P, 2 * W], F32, kind="ExternalOutput"),
            "dbg_se": nc.dram_tensor("dbg_se", [P, T], F32, kind="ExternalOutput"),
            "dbg_mo": nc.dram_tensor("dbg_mo", [P, 2 * W], F32, kind="ExternalOutput"),
            "dbg_ch": nc.dram_tensor("dbg_ch", [1, 16 * W], F32, kind="ExternalOutput"),
        }

    with tile.TileContext(nc) as tc:
        _body(nc, tc, cband, los, fT8_d, fg8_d, ohb_d, mo8s_d, memf_d,
              out_d, dbg)
    nc.compile()
    return nc


def _body(nc, tc, CB, los, fT8_d, fg8_d, ohb_d, mo8s_d, memf_d, out_d,
          dbg=None):
    with tc.tile_pool(name="const", bufs=1) as cpool, \
         tc.tile_pool(name="junk", bufs=8) as jpool, \
         tc.tile_pool(name="dram", bufs=1, space="DRAM") as dpool:

        # ---------------- persistent SBUF tiles ----------------
        fT8a = cpool.tile([P, 2, 2048], FP8, tag="fT8a")
        fT8b = cpool.tile([P, 2, R - 2048], FP8, tag="fT8b")
        fg8 = cpool.tile([P, TP, 2, D], FP8, tag="fg8")
        ohb = cpool.tile([P, TP, 2, CB], FP8, tag="ohb")
        mo8s = cpool.tile([P, 2, W], FP8, tag="mo8s")
        mo8m = cpool.tile([P, 2, W], FP8, tag="mo8m")
        memf = cpool.tile([P, 2, W], BF16, tag="memf")
        Sg = cpool.tile([P, 2, W], BF16, tag="Sg")
        gath = cpool.tile([P, N_CORES, 2, CB], FP8, tag="gath")
        q = cpool.tile([P, 2, 2, W], BF16, tag="q")
        ch = cpool.tile([1, 16 * W], BF16, tag="chain")
        ab = cpool.tile([1, 2 * W], BF16, tag="ab")

        se_a = [cpool.tile([P, H], F32, tag=f"se_a{i}", name=f"se_a{i}")
                for i in range(2)]
        se_b = [cpool.tile([P, H], F32, tag=f"se_b{i}", name=f"se_b{i}")
                for i in range(2)]
        se = [cpool.tile([P, H], F32, tag=f"se{i}", name=f"se{i}")
                for i in range(2)]
        zbuf = [cpool.tile([P, H], F32, tag=f"zbuf{i}", name=f"zbuf{i}")
                for i in range(2)]
        zsum2 = cpool.tile([P, 2], F32, tag="zsum2")
        zsum = cpool.tile([P, 1], F32, tag="zsum")
        zred = cpool.tile([P, 1], F32, tag="zred")
        dotp = cpool.tile([1, 2], F32, tag="dotp")
        outrow = cpool.tile([1, 2], F32, tag="outrow")

        ebias = cpool.tile([P, 1], F32, tag="ebias")
        ones_col = cpool.tile([P, 1], BF16, tag="ones_col")
        ones_row = cpool.tile([1, P], BF16, tag="ones_row")
        wtile = cpool.tile([P, 2, 512], FP8, tag="wtile")
        dj = cpool.tile([P, 1], F32, tag="dj")

        nc.vector.memset(ebias[:], EPS * EPS)
        nc.vector.memset(ones_col[:], 1.0)
        nc.vector.memset(ones_row[:], 1.0)
        nc.gpsimd.memset(wtile[:], 0.0)
        # prime the exp/ln table set once, early
        nc.scalar.activation(dj[:], ebias[:], AF.Exp, bias=ebias[:])
        nc.scalar.activation(dj[:], ebias[:], AF.Ln, bias=ebias[:])

        # -------- input DMAs: 2 issue queues so fixed overheads overlap --
        fT8r = fT8_d.ap().rearrange("p (k r) -> p k r", k=2)
        fg8r = fg8_d.ap().rearrange("p (t k d) -> p t k d", t=TP, k=2)
        ohbr = ohb_d.ap().rearrange("p (t k c) -> p t k c", t=TP, k=2)
        # sync: tiny critical loads, then the one-hot band block
        nc.sync.dma_start(mo8s[:], mo8s_d.ap().rearrange("p (k c) -> p k c", k=2))
        nc.sync.dma_start(fT8a[:], fT8r[:, :, 0:2048])
        nc.sync.dma_start(ohb[:], ohbr[:])
        # gpsimd: bulk loads (wtile memset precedes, Sg memset follows)
        nc.gpsimd.dma_start(fg8[:], fg8r[:])
        nc.gpsimd.dma_start(memf[:], memf_d.ap().rearrange("p (k c) -> p k c", k=2))
        nc.gpsimd.dma_start(fT8b[:], fT8r[:, :, 2048:R])
        nc.gpsimd.memset(Sg[:], 0.0)

        def ftile(t):
            if t < 16:
                return fT8a[:, :, t * P:(t + 1) * P]
            return fT8b[:, :, (t - 16) * P:(t - 15) * P]

        # ---------------- a2 (source-half) tile machinery --------------
        a2_state = {"next": 0}

        def emit_a2(pool, n=1):
            for _ in range(n):
                t = a2_state["next"]
                if t >= T:
                    return
                a2_state["next"] = t + 1
                ps = pool.tile([P, 1024], F32, tag="a2", name=f"a2_{t}")
                for c0, c1 in ((0, 512), (512, W)):
                    nc.tensor.matmul(
                        ps[:, c0:c1],
                        lhsT=ftile(t),
                        rhs=mo8s[:, :, c0:c1],
                        start=True, stop=True, perf_mode=DR)
                ej = jpool.tile([P, W], BF16, tag="ej", name=f"ej{t}")
                nc.scalar.activation(ej[:], ps[:, 0:W], AF.Exp)
                nc.vector.tensor_scalar(
                    ej[:], ej[:], 0.0, 0.0, ALU.add, ALU.add,
                    accum_out=se_a[t // H][:, t % H:t % H + 1])

        # =============== stage SS + AllGather (+ a2 stream) =============
        gout = None
        with tc.tile_pool(name="a2ps", bufs=2, space="PSUM") as a2pool:
            # warmup: ramp the PE pstate before real work lands
            wp = a2pool.tile([P, 1024], F32, tag="a2", name="warm")
            for i in range(8):
                nc.tensor.matmul(wp[:, 0:512], lhsT=wtile[:, :, 0:P],
                                 rhs=wtile[:],
                                 start=(i == 0), stop=(i == 7), perf_mode=DR)

            with tc.tile_pool(name="ssps", bufs=1, space="PSUM") as ssps:
                ps_ss = [ssps.tile([P, CB], F32, tag=f"ss{h}", name=f"ss{h}")
                         for h in range(2)]
                emit_a2(a2pool, 2)
                for tp in range(TP):
                    for h in range(2):
                        for c0 in range(0, CB, 512):
                            c1 = min(c0 + 512, CB)
                            nc.tensor.matmul(
                                ps_ss[h][:, c0:c1],
                                lhsT=fg8[:, tp, :, h * P:(h + 1) * P],
                                rhs=ohb[:, tp, :, c0:c1],
                                start=(tp == 0), stop=(tp == TP - 1),
                                perf_mode=DR)
                    if a2_state["next"] < min(2 * (tp + 2), A2_DUMP):
                        emit_a2(a2pool, 1)
                slband = dpool.tile([2 * P, CB], FP8, tag="slband")
                for h in range(2):
                    dmp = cpool.tile([P, CB], FP8, tag=f"dump{h}",
                                     name=f"dump{h}")
                    nc.vector.tensor_copy(dmp[:], ps_ss[h][:])
                    nc.gpsimd.dma_start(slband[h * P:(h + 1) * P, :], dmp[:])
                gout = dpool.tile([N_CORES * 2 * P, CB], FP8, tag="gout",
                                  addr_space="Shared")
                nc.gpsimd.collective_compute(
                    "AllGather", ALU.bypass,
                    replica_groups=[list(range(N_CORES))],
                    ins=[slband.opt()], outs=[gout.opt()])

            # bring the 8 bands in and rebuild global S (bf16)
            nc.gpsimd.dma_start(
                gath[:], gout[:].rearrange("(g h p) c -> p g h c", g=N_CORES,
                                             h=2, p=P))
            emit_a2(a2pool, A2_SS + 1 - a2_state["next"])
            for k in range(N_CORES):
                lo = los[k]
                nc.vector.tensor_tensor(
                    Sg[:, :, lo:lo + CB], Sg[:, :, lo:lo + CB],
                    gath[:, k, :, :], ALU.add)
            # q = [S*S | S*mem] for both ko halves in single strided ops
            nc.vector.tensor_tensor(q[:, :, 0, :], Sg[:], Sg[:], ALU.mult)
            nc.vector.tensor_tensor(q[:, :, 1, :], Sg[:], memf[:], ALU.mult)

            # =============== stage NM (new memory) ======================
            with tc.tile_pool(name="nmps", bufs=1, space="PSUM") as nmps:
                ps_nw = nmps.tile([1, 2048], F32, tag="nw", name="ps_nw")
                for j in range(2):
                    for c0, c1 in ((0, 512), (512, W)):
                        for ko in range(2):
                            nc.tensor.matmul(
                                ps_nw[:, j * 1024 + c0:j * 1024 + c1],
                                lhsT=ones_col[:],
                                rhs=q[:, ko, j, c0:c1],
                                start=(ko == 0), stop=(ko == 1))
                nsq = ps_nw[:, 0:W]
                wraw = ps_nw[:, 1024:1024 + W]

                lnn, invn, w_, g, g2, g3, gd, n2, ln2, inv2, u = (
                    ch[:, i * W:(i + 1) * W] for i in range(11))
                emit_a2(a2pool, A2_LNN - a2_state["next"])
                # invn = 1/sqrt(nsq+eps^2) = exp(-0.5*ln(nsq+eps^2))
                nc.scalar.activation(lnn, nsq, AF.Ln, bias=ebias[0:1, :])
                nc.scalar.activation(invn, lnn, AF.Exp, scale=-0.5)

                nc.vector.tensor_tensor(w_, wraw, invn, ALU.mult)
                nc.vector.tensor_scalar(g, w_, -1.0, 1.0, ALU.mult, ALU.add)
                nc.vector.tensor_tensor(g2, g, g, ALU.mult)
                nc.vector.tensor_tensor(g3, g2, g, ALU.mult)
                nc.vector.tensor_tensor(gd, g3, g2, ALU.subtract)
                # n2 = |w*mem + g*bc|^2 = 1 + 2(g^3 - g^2)
                nc.vector.tensor_scalar(n2, gd, 2.0, 1.0, ALU.mult, ALU.add)
                emit_a2(a2pool, A2_CH - a2_state["next"])
                nc.scalar.activation(ln2, n2, AF.Ln, bias=ebias[0:1, :])
                nc.scalar.activation(inv2, ln2, AF.Exp, scale=-0.5)

                nc.vector.tensor_tensor(u, g, invn, ALU.mult)
                nc.vector.tensor_tensor(ab[:, 0:W], inv2, w_, ALU.mult)
                nc.vector.tensor_tensor(ab[:, W:2 * W], inv2, u, ALU.mult)
                # dot = <S, new_m> = <a, wraw> + <b, nsq> via stt accum
                dj1 = ch[:, 11 * W:12 * W]
                dj2 = ch[:, 12 * W:13 * W]
                nc.vector.scalar_tensor_tensor(
                    out=dj1, in0=wraw, scalar=1.0, in1=ab[:, 0:W],
                    op0=ALU.mult, op1=ALU.mult, accum_out=dotp[:, 0:1])
                nc.vector.scalar_tensor_tensor(
                    out=dj2, in0=nsq, scalar=1.0, in1=ab[:, W:2 * W],
                    op0=ALU.mult, op1=ALU.mult, accum_out=dotp[:, 1:2])

            # broadcast a/b to 128 partitions with a K=1 matmul, then
            # new_m = a*mem + b*S  (fp8, feeds the memory-half matmuls)
            with tc.tile_pool(name="abps", bufs=1, space="PSUM") as abps:
                abbc = abps.tile([P, 2048], F32, tag="abbc", name="abbc")
                for j in range(2):
                    for c0, c1 in ((0, 512), (512, W)):
                        nc.tensor.matmul(
                            abbc[:, j * 1024 + c0:j * 1024 + c1],
                            lhsT=ones_row[:], rhs=ab[:, j * W + c0:j * W + c1],
                            start=True, stop=True)
                emit_a2(a2pool, A2_MO - a2_state["next"])
                for ko in range(2):
                    t1 = jpool.tile([P, W], BF16, tag="t12", name=f"t1{ko}")
                    t2 = jpool.tile([P, W], BF16, tag="t12", name=f"t2{ko}")
                    nc.vector.tensor_tensor(t1[:], memf[:, ko, :],
                                            abbc[:, 0:W], ALU.mult)
                    nc.vector.tensor_tensor(t2[:], Sg[:, ko, :],
                                            abbc[:, 1024:1024 + W], ALU.mult)
                    nc.vector.tensor_tensor(mo8m[:, ko, :], t1[:], t2[:],
                                            ALU.add)
                emit_a2(a2pool, T - a2_state["next"])

        # =============== memory-half tiles ==============================
        def b_half(pool, t0, nt):
            ps = pool.tile([P, 2048], F32, tag="lgf", name=f"b{t0}")
            for j in range(nt):
                t = t0 + j
                for c0, c1 in ((0, 512), (512, W)):
                    nc.tensor.matmul(
                        ps[:, j * 1024 + c0:j * 1024 + c1],
                        lhsT=ftile(t),
                        rhs=mo8m[:, :, c0:c1],
                        start=True, stop=True, perf_mode=DR)
            ej = jpool.tile([P, 2 * W], BF16, tag="ejb", name=f"ejb{t0}")
            psv = ps[:].rearrange("p (j c) -> p j c", j=2)[:, 0:nt, 0:W]
            nc.scalar.activation(ej[:, 0:nt * W], psv, AF.Exp)
            for j in range(nt):
                t = t0 + j
                nc.vector.tensor_scalar(
                    ej[:, j * W:(j + 1) * W], ej[:, j * W:(j + 1) * W],
                    0.0, 0.0, ALU.add, ALU.add,
                    accum_out=se_b[t // H][:, t % H:t % H + 1])

        with tc.tile_pool(name="lgF", bufs=2, space="PSUM") as lgF:
            done = 0
            while done < B_SINGLES:
                b_half(lgF, done, 1)
                done += 1
            while done < T:
                nt = min(2, T - done)
                b_half(lgF, done, nt)
                done += nt
                # first half ready -> fold + ln early
                if done == H + 4:
                    nc.vector.tensor_tensor(se[0][:], se_a[0][:], se_b[0][:],
                                            ALU.add)
                    nc.scalar.activation(zbuf[0][:], se[0][:], AF.Ln,
                                         accum_out=zsum2[:, 0:1])

        # =============== finalize ======================================
        nc.vector.tensor_tensor(se[1][:], se_a[1][:], se_b[1][:], ALU.add)
        nc.scalar.activation(zbuf[1][:], se[1][:], AF.Ln,
                             accum_out=zsum2[:, 1:2])
        nc.vector.tensor_tensor(zsum[:], zsum2[:, 0:1], zsum2[:, 1:2], ALU.add)
        nc.gpsimd.partition_all_reduce(zred[:], zsum[:], P,
                                       bass_isa.ReduceOp.add)
        nc.vector.tensor_copy(outrow[:, 0:1], zred[0:1, :])
        nc.vector.tensor_tensor(outrow[:, 1:2], dotp[:, 0:1], dotp[:, 1:2],
                                ALU.add)
        nc.sync.dma_start(out_d.ap(), outrow[:])

        if dbg is not None:
            sgf = cpool.tile([P, 2 * W], F32, tag="sgf")
            nc.vector.tensor_copy(sgf[:], Sg[:].rearrange("p k c -> p (k c)"))
            nc.sync.dma_start(dbg["dbg_sg"].ap(), sgf[:])
            chf = cpool.tile([1, 16 * W], F32, tag="chf")
            nc.vector.tensor_copy(chf[:], ch[:])
            nc.sync.dma_start(dbg["dbg_ch"].ap(), chf[:])
            mof = cpool.tile([P, 2 * W], F32, tag="mof")
            nc.vector.tensor_copy(mof[:], mo8m[:].rearrange("p k c -> p (k c)"))
            nc.sync.dma_start(dbg["dbg_mo"].ap(), mof[:])
            sef = cpool.tile([P, T], F32, tag="sef")
            nc.vector.tensor_copy(sef[:, 0:H], se[0][:])
            nc.vector.tensor_copy(sef[:, H:T], se[1][:])
            nc.sync.dma_start(dbg["dbg_se"].ap(), sef[:])


def _dpair(mat_cd, dtype):
    """[C, D] -> [P, 2, C] with tile[ki, ko, c] = M[c, ko*128+ki]."""
    mt = mat_cd.T.reshape(2, P, mat_cd.shape[0])  # [ko, ki, c]
    return np.ascontiguousarray(mt.transpose(1, 0, 2).astype(dtype))


def _prep_inputs(feat, label, memory, source_memo):
    feat = np.asarray(feat, dtype=np.float32)
    label = np.asarray(label).astype(np.int64)
    memory = np.asarray(memory, dtype=np.float32)
    source_memo = np.asarray(source_memo, dtype=np.float32)

    nrm = np.maximum(np.sqrt((feat * feat).sum(axis=1, keepdims=True)),
                     np.float32(EPS))
    fn = (feat / nrm).astype(np.float32)

    order = np.argsort(label, kind="stable")
    fs_all = fn[order]
    ls_all = label[order]

    # per-core class bands (compile-time constants, shared SPMD program)
    los, spans = [], []
    for k in range(N_CORES):
        lk = int(ls_all[k * R])
        hk = int(ls_all[(k + 1) * R - 1])
        los.append(lk)
        spans.append(hk - lk + 1)
    cband = min(-(-max(spans) // 256) * 256, W)
    los = [min(lo, W - cband) for lo in los]

    mo8s = _dpair(source_memo, ml_dtypes.float8_e4m3fn).reshape(P, -1)
    memf = _dpair(memory, ml_dtypes.bfloat16).reshape(P, -1)

    in_maps = []
    for k in range(N_CORES):
        fs = fs_all[k * R:(k + 1) * R]
        ls = ls_all[k * R:(k + 1) * R]
        f4 = fs.reshape(TP, 2, P, D)               # [tp, ko, ki, d]
        fg8 = f4.transpose(2, 0, 1, 3).reshape(P, TP * 2 * D)
        fT8 = fs.T.reshape(2, P, R).transpose(1, 0, 2).reshape(P, 2 * R)
        rel = (ls - los[k]).reshape(TP, 2, P)       # [tp, ko, ki]
        oh4 = (rel[..., None] == np.arange(cband)[None, None, None, :])
        ohb = oh4.transpose(2, 0, 1, 3).reshape(P, TP * 2 * cband)
        in_maps.append({
            "fT8": np.ascontiguousarray(fT8.astype(ml_dtypes.float8_e4m3fn)),
            "fg8": np.ascontiguousarray(fg8.astype(ml_dtypes.float8_e4m3fn)),
            "ohb": np.ascontiguousarray(ohb.astype(ml_dtypes.float8_e4m3fn)),
            "mo8s": mo8s,
            "memf": memf,
        })
    return in_maps, cband, los


def _install_trace_hook():
    """The image's antenv lacks axon_hooks; recreate it from trn_agent_boot."""
    import sys, types
    import antenv
    if "antenv.axon_hooks" in sys.modules:
        return
    from trn_agent_boot.trn_boot import _ntff_profile_via_ctypes
    hook = _ntff_profile_via_ctypes("/opt/axon/libaxon_pjrt.so")
    m = types.ModuleType("antenv.axon_hooks")
    m.get_axon_ntff_profile_hook = lambda: hook
    sys.modules["antenv.axon_hooks"] = m
    antenv.axon_hooks = m
    import concourse.bass_utils as bu
    bu.upload_artifacts = lambda tmpdir: tmpdir


def _run(feat, label, memory, source_memo, trace=False, debug=False):
    if trace:
        _install_trace_hook()
    in_maps, cband, los = _prep_inputs(feat, label, memory, source_memo)
    key = (cband, tuple(los), debug)
    if key not in _CACHE:
        _CACHE[key] = _build(cband, los, debug)
    nc = _CACHE[key]
    res = run_bass_kernel_spmd(nc, in_maps, list(range(N_CORES)), trace=trace)
    zsum_total = sum(float(res.results[i]["out"][0, 0]) for i in range(N_CORES))
    dot = float(res.results[0]["out"][0, 1])
    loss = (zsum_total - dot) / N_TOTAL
    return np.asarray(loss, dtype=np.float32), res


def kernel(feat, label, memory, source_memo):
    loss, _ = _run(feat, label, memory, source_memo, trace=False)
    return loss


# revision 24
# speedup vs baseline: 1.0670x; 1.0670x over previous
"""Trainium2 Bass kernel v3 for scatter_memory (nn_Memory_90031104459201).

Math (per reference.py):
    feat_n = l2norm(feat)                         [65536, 256]
    S      = segment_sum(feat_n, label, 1000)     [1000, 256]
    bc     = l2norm(S); w = <mem, bc>
    new_m  = l2norm(w*mem + (1-w)*bc)
    logits = feat_n @ [new_m; src].T              [65536, 2000]
    loss   = mean(logsumexp(logits)) - <S, new_m>_F / 65536

v3 strategy (8 cores, data-parallel rows, 8192 rows/core):
  - HOST SORTS ROWS BY LABEL (loss is row-permutation invariant): each
    core's 8192 rows then cover a ~125-class band.  The one-hot for the
    segment-sum shrinks from [8192,1024] to [8192,CBAND] (CBAND~192),
    i.e. 1.5 MiB instead of 8 MiB of DMA, and the segment-sum matmul
    writes only a [128, CBAND] psum band.
  - Cross-core reduction becomes an AllGather of the 8 per-core bands
    (cost model: 15us constant, vs AllReduce 15us*1.875) + 8 bf16
    adds on DVE to reconstruct the global S.
  - new_memory in closed form with g=1-w (no flags: empty classes give
    w=0 naturally); rsqrt via exp(-0.5*ln(x)) so ACT keeps ONE table
    set; per-class a/b coefs broadcast to 128 partitions via a K=1
    matmul; <S,new_m> via two stt accum_out ops.
  - Logits row-tiles of 128: the 64 source-half tiles [128,1000] are
    fully independent and keep ACT busy from ~5us while the collective
    + chain complete; memory-half tiles run after new_m is ready,
    paired two-at-a-time [128,2048].  All row-sums of exp() are DVE
    tensor_reduce (no ACT accumulator reads).  ACT is the bottleneck
    engine at ~122us busy.
  - CBAND and the 8 band offsets are input-dependent compile constants
    (same for all cores -> single SPMD program); any label distribution
    just changes the constants, degenerating gracefully to CBAND=1000.
"""

import numpy as np
import ml_dtypes

import concourse.bass as bass
import concourse.bass_isa as bass_isa
import concourse.mybir as mybir
import concourse.tile as tile
from concourse import bacc
from concourse.bass_utils import run_bass_kernel_spmd

F32 = mybir.dt.float32
BF16 = mybir.dt.bfloat16
F16 = mybir.dt.float16
FP8 = mybir.dt.float8e4
AF = mybir.ActivationFunctionType
ALU = mybir.AluOpType
DR = mybir.MatmulPerfMode.DoubleRow

N_CORES = 8
N_TOTAL = 65536
R = N_TOTAL // N_CORES   # 8192 rows/core
D = 256
C = 1000
P = 128
TP = 32                  # row-pair tiles (256 rows each)
T = 64                   # logits row tiles of 128
W = 1000                 # class width per half (exact, no padding)
H = T // 2
EPS = 1e-12

# schedule knobs: a2 = source-half logit tiles (ACT filler work).
# Segment boundaries: how many a2 tiles are emitted before each stage
# of the NM chain goes into the (in-order) engine programs.
A2_DUMP = 8              # a2 tiles interleaved before the dump copies
A2_SS = 20               # a2 tiles emitted by the end of the ss phase
A2_LNN = 38              # before the invn ln/exp pair
A2_CH = 42               # before the chain DVE block + ln2/exp2
A2_MO = 45               # before abbc/mo8 writes; rest after
B_SINGLES = 0            # memory-half tiles done singly at the seam

_CACHE = {}


def _patch_act_tables():
    """Map exp/ln to the combined natural_log_exp_and_others set so the
    ACT engine loads its spline tables exactly once."""
    import concourse.bacc as bacc_mod
    if getattr(bacc_mod, "_act_tables_patched", False):
        return
    orig = bacc_mod.get_activation_tables

    def patched(arch):
        tabs = orig(arch)
        combined = "natural_log_exp_and_others"
        if combined in tabs:
            keep = tabs[combined]
            tabs = {k: (v if k == combined else (v - keep))
                    for k, v in tabs.items()}
        return tabs

    bacc_mod.get_activation_tables = patched
    bacc_mod._act_tables_patched = True


def _build(cband, los, debug=False):
    _patch_act_tables()
    nc = bacc.Bacc("TRN2", num_devices=N_CORES)

    fT8_d = nc.dram_tensor("fT8", [P, 2 * R], FP8, kind="ExternalInput")
    fg8_d = nc.dram_tensor("fg8", [P, TP * 2 * D], FP8, kind="ExternalInput")
    ohb_d = nc.dram_tensor("ohb", [P, TP * 2 * cband], FP8, kind="ExternalInput")
    mo8s_d = nc.dram_tensor("mo8s", [P, 2 * W], FP8, kind="ExternalInput")
    memf_d = nc.dram_tensor("memf", [P, 2 * W], BF16, kind="ExternalInput")
    out_d = nc.dram_tensor("out", [1, 2], F32, kind="ExternalOutput")
    dbg = None
    if debug:
        dbg = {
            "dbg_sg": nc.dram_tensor("dbg_sg", [P, 2 * W], F32, kind="ExternalOutput"),
            "dbg_se": nc.dram_tensor("dbg_se", [P, T], F32, kind="ExternalOutput"),
            "dbg_mo": nc.dram_tensor("dbg_mo", [P, 2 * W], F32, kind="ExternalOutput"),
            "dbg_ch": nc.dram_tensor("dbg_ch", [1, 16 * W], F32, kind="ExternalOutput"),
        }

    with tile.TileContext(nc) as tc:
        _body(nc, tc, cband, los, fT8_d, fg8_d, ohb_d, mo8s_d, memf_d,
              out_d, dbg)
    nc.compile()
    return nc


def _body(nc, tc, CB, los, fT8_d, fg8_d, ohb_d, mo8s_d, memf_d, out_d,
          dbg=None):
    with tc.tile_pool(name="const", bufs=1) as cpool, \
         tc.tile_pool(name="junk", bufs=8) as jpool, \
         tc.tile_pool(name="dram", bufs=1, space="DRAM") as dpool:

        # ---------------- persistent SBUF tiles ----------------
        fT8a = cpool.tile([P, 2, 2048], FP8, tag="fT8a")
        fT8b = cpool.tile([P, 2, R - 2048], FP8, tag="fT8b")
        fg8 = cpool.tile([P, TP, 2, D], FP8, tag="fg8")
        ohb = cpool.tile([P, TP, 2, CB], FP8, tag="ohb")
        mo8s = cpool.tile([P, 2, W], FP8, tag="mo8s")
        mo8m = cpool.tile([P, 2, W], FP8, tag="mo8m")
        memf = cpool.tile([P, 2, W], BF16, tag="memf")
        Sg = cpool.tile([P, 2, W], BF16, tag="Sg")
        gath = cpool.tile([P, N_CORES, 2, CB], FP8, tag="gath")
        q = cpool.tile([P, 2, 2, W], BF16, tag="q")
        ch = cpool.tile([1, 16 * W], BF16, tag="chain")
        ab = cpool.tile([1, 2 * W], BF16, tag="ab")

        se_a = [cpool.tile([P, H], F32, tag=f"se_a{i}", name=f"se_a{i}")
                for i in range(2)]
        se_b = [cpool.tile([P, H], F32, tag=f"se_b{i}", name=f"se_b{i}")
                for i in range(2)]
        se = [cpool.tile([P, H], F32, tag=f"se{i}", name=f"se{i}")
                for i in range(2)]
        zbuf = [cpool.tile([P, H], F32, tag=f"zbuf{i}", name=f"zbuf{i}")
                for i in range(2)]
        zsum2 = cpool.tile([P, 2], F32, tag="zsum2")
        zsum = cpool.tile([P, 1], F32, tag="zsum")
        zred = cpool.tile([P, 1], F32, tag="zred")
        dotp = cpool.tile([1, 2], F32, tag="dotp")
        outrow = cpool.tile([1, 2], F32, tag="outrow")

        ebias = cpool.tile([P, 1], F32, tag="ebias")
        ones_col = cpool.tile([P, 1], BF16, tag="ones_col")
        ones_row = cpool.tile([1, P], BF16, tag="ones_row")
        wtile = cpool.tile([P, 2, 512], FP8, tag="wtile")
        dj = cpool.tile([P, 1], F32, tag="dj")

        nc.vector.memset(ebias[:], EPS * EPS)
        nc.vector.memset(ones_col[:], 1.0)
        nc.vector.memset(ones_row[:], 1.0)
        nc.gpsimd.memset(wtile[:], 0.0)
        # prime the exp/ln table set once, early
        nc.scalar.activation(dj[:], ebias[:], AF.Exp, bias=ebias[:])
        nc.scalar.activation(dj[:], ebias[:], AF.Ln, bias=ebias[:])

        # -------- input DMAs: 2 issue queues so fixed overheads overlap --
        fT8r = fT8_d.ap().rearrange("p (k r) -> p k r", k=2)
        fg8r = fg8_d.ap().rearrange("p (t k d) -> p t k d", t=TP, k=2)
        ohbr = ohb_d.ap().rearrange("p (t k c) -> p t k c", t=TP, k=2)
        # sync: tiny critical loads, then the one-hot band block
        nc.sync.dma_start(mo8s[:], mo8s_d.ap().rearrange("p (k c) -> p k c", k=2))
        nc.sync.dma_start(fT8a[:], fT8r[:, :, 0:2048])
        nc.sync.dma_start(ohb[:], ohbr[:])
        # gpsimd: bulk loads (wtile memset precedes, Sg memset follows)
        nc.gpsimd.dma_start(fg8[:], fg8r[:])
        nc.gpsimd.dma_start(memf[:], memf_d.ap().rearrange("p (k c) -> p k c", k=2))
        nc.gpsimd.dma_start(fT8b[:], fT8r[:, :, 2048:R])
        nc.gpsimd.memset(Sg[:], 0.0)

        def ftile(t):
            if t < 16:
                return fT8a[:, :, t * P:(t + 1) * P]
            return fT8b[:, :, (t - 16) * P:(t - 15) * P]

        # ---------------- a2 (source-half) tile machinery --------------
        a2_state = {"next": 0}

        def emit_a2(pool, n=1):
            for _ in range(n):
                t = a2_state["next"]
                if t >= T:
                    return
                a2_state["next"] = t + 1
                ps = pool.tile([P, 1024], F32, tag="a2", name=f"a2_{t}")
                for c0, c1 in ((0, 512), (512, W)):
                    nc.tensor.matmul(
                        ps[:, c0:c1],
                        lhsT=ftile(t),
                        rhs=mo8s[:, :, c0:c1],
                        start=True, stop=True, perf_mode=DR)
                ej = jpool.tile([P, W], BF16, tag="ej", name=f"ej{t}")
                nc.scalar.activation(ej[:], ps[:, 0:W], AF.Exp)
                nc.vector.tensor_scalar(
                    ej[:], ej[:], 0.0, 0.0, ALU.add, ALU.add,
                    accum_out=se_a[t // H][:, t % H:t % H + 1])

        # =============== stage SS + AllGather (+ a2 stream) =============
        gout = None
        with tc.tile_pool(name="a2ps", bufs=2, space="PSUM") as a2pool:
            # warmup: ramp the PE pstate before real work lands
            wp = a2pool.tile([P, 1024], F32, tag="a2", name="warm")
            for i in range(8):
                nc.tensor.matmul(wp[:, 0:512], lhsT=wtile[:, :, 0:P],
                                 rhs=wtile[:],
                                 start=(i == 0), stop=(i == 7), perf_mode=DR)

            with tc.tile_pool(name="ssps", bufs=1, space="PSUM") as ssps:
                ps_ss = [ssps.tile([P, CB], F32, tag=f"ss{h}", name=f"ss{h}")
                         for h in range(2)]
                emit_a2(a2pool, 2)
                for tp in range(TP):
                    for h in range(2):
                        for c0 in range(0, CB, 512):
                            c1 = min(c0 + 512, CB)
                            nc.tensor.matmul(
                                ps_ss[h][:, c0:c1],
                                lhsT=fg8[:, tp, :, h * P:(h + 1) * P],
                                rhs=ohb[:, tp, :, c0:c1],
                                start=(tp == 0), stop=(tp == TP - 1),
                                perf_mode=DR)
                    if a2_state["next"] < min(2 * (tp + 2), A2_DUMP):
                        emit_a2(a2pool, 1)
                slband = dpool.tile([2 * P, CB], FP8, tag="slband")
                for h in range(2):
                    dmp = cpool.tile([P, CB], FP8, tag=f"dump{h}",
                                     name=f"dump{h}")
                    nc.vector.tensor_copy(dmp[:], ps_ss[h][:])
                    nc.gpsimd.dma_start(slband[h * P:(h + 1) * P, :], dmp[:])
                gout = dpool.tile([N_CORES * 2 * P, CB], FP8, tag="gout",
                                  addr_space="Shared")
                nc.gpsimd.collective_compute(
                    "AllGather", ALU.bypass,
                    replica_groups=[list(range(N_CORES))],
                    ins=[slband.opt()], outs=[gout.opt()])

            # bring the 8 bands in and rebuild global S (bf16)
            nc.gpsimd.dma_start(
                gath[:], gout[:].rearrange("(g h p) c -> p g h c", g=N_CORES,
                                             h=2, p=P))
            emit_a2(a2pool, A2_SS + 1 - a2_state["next"])
            for k in range(N_CORES):
                lo = los[k]
                nc.vector.tensor_tensor(
                    Sg[:, :, lo:lo + CB], Sg[:, :, lo:lo + CB],
                    gath[:, k, :, :], ALU.add)
            # q = [S*S | S*mem] for both ko halves in single strided ops
            nc.vector.tensor_tensor(q[:, :, 0, :], Sg[:], Sg[:], ALU.mult)
            nc.vector.tensor_tensor(q[:, :, 1, :], Sg[:], memf[:], ALU.mult)

            # =============== stage NM (new memory) ======================
            with tc.tile_pool(name="nmps", bufs=1, space="PSUM") as nmps:
                ps_nw = nmps.tile([1, 2048], F32, tag="nw", name="ps_nw")
                for j in range(2):
                    for c0, c1 in ((0, 512), (512, W)):
                        for ko in range(2):
                            nc.tensor.matmul(
                                ps_nw[:, j * 1024 + c0:j * 1024 + c1],
                                lhsT=ones_col[:],
                                rhs=q[:, ko, j, c0:c1],
                                start=(ko == 0), stop=(ko == 1))
                nsq = ps_nw[:, 0:W]
                wraw = ps_nw[:, 1024:1024 + W]

                lnn, invn, w_, g, g2, g3, gd, n2, ln2, inv2, u = (
                    ch[:, i * W:(i + 1) * W] for i in range(11))
                emit_a2(a2pool, A2_LNN - a2_state["next"])
                # invn = 1/sqrt(nsq+eps^2) = exp(-0.5*ln(nsq+eps^2))
                nc.scalar.activation(lnn, nsq, AF.Ln, bias=ebias[0:1, :])
                nc.scalar.activation(invn, lnn, AF.Exp, scale=-0.5)

                nc.vector.tensor_tensor(w_, wraw, invn, ALU.mult)
                nc.vector.tensor_scalar(g, w_, -1.0, 1.0, ALU.mult, ALU.add)
                nc.vector.tensor_tensor(g2, g, g, ALU.mult)
                nc.vector.tensor_tensor(g3, g2, g, ALU.mult)
                nc.vector.tensor_tensor(gd, g3, g2, ALU.subtract)
                # n2 = |w*mem + g*bc|^2 = 1 + 2(g^3 - g^2)
                nc.vector.tensor_scalar(n2, gd, 2.0, 1.0, ALU.mult, ALU.add)
                emit_a2(a2pool, A2_CH - a2_state["next"])
                nc.scalar.activation(ln2, n2, AF.Ln, bias=ebias[0:1, :])
                nc.scalar.activation(inv2, ln2, AF.Exp, scale=-0.5)

                nc.vector.tensor_tensor(u, g, invn, ALU.mult)
                nc.vector.tensor_tensor(ab[:, 0:W], inv2, w_, ALU.mult)
                nc.vector.tensor_tensor(ab[:, W:2 * W], inv2, u, ALU.mult)
                # dot = <S, new_m> = <a, wraw> + <b, nsq> via stt accum
                dj1 = ch[:, 11 * W:12 * W]
                dj2 = ch[:, 12 * W:13 * W]
                nc.vector.scalar_tensor_tensor(
                    out=dj1, in0=wraw, scalar=1.0, in1=ab[:, 0:W],
                    op0=ALU.mult, op1=ALU.mult, accum_out=dotp[:, 0:1])
                nc.vector.scalar_tensor_tensor(
                    out=dj2, in0=nsq, scalar=1.0, in1=ab[:, W:2 * W],
                    op0=ALU.mult, op1=ALU.mult, accum_out=dotp[:, 1:2])

            # broadcast a/b to 128 partitions with a K=1 matmul, then
            # new_m = a*mem + b*S  (fp8, feeds the memory-half matmuls)
            with tc.tile_pool(name="abps", bufs=1, space="PSUM") as abps:
                abbc = abps.tile([P, 2048], F32, tag="abbc", name="abbc")
                for j in range(2):
                    for c0, c1 in ((0, 512), (512, W)):
                        nc.tensor.matmul(
                            abbc[:, j * 1024 + c0:j * 1024 + c1],
                            lhsT=ones_row[:], rhs=ab[:, j * W + c0:j * W + c1],
                            start=True, stop=True)
                emit_a2(a2pool, A2_MO - a2_state["next"])
                for ko in range(2):
                    t1 = jpool.tile([P, W], BF16, tag="t12", name=f"t1{ko}")
                    t2 = jpool.tile([P, W], BF16, tag="t12", name=f"t2{ko}")
                    nc.vector.tensor_tensor(t1[:], memf[:, ko, :],
                                            abbc[:, 0:W], ALU.mult)
                    nc.vector.tensor_tensor(t2[:], Sg[:, ko, :],
                                            abbc[:, 1024:1024 + W], ALU.mult)
                    nc.vector.tensor_tensor(mo8m[:, ko, :], t1[:], t2[:],
                                            ALU.add)
                emit_a2(a2pool, T - a2_state["next"])

        # =============== memory-half tiles ==============================
        def b_half(pool, t0, nt):
            ps = pool.tile([P, 2048], F32, tag="lgf", name=f"b{t0}")
            for j in range(nt):
                t = t0 + j
                for c0, c1 in ((0, 512), (512, W)):
                    nc.tensor.matmul(
                        ps[:, j * 1024 + c0:j * 1024 + c1],
                        lhsT=ftile(t),
                        rhs=mo8m[:, :, c0:c1],
                        start=True, stop=True, perf_mode=DR)
            ej = jpool.tile([P, 2 * W], BF16, tag="ejb", name=f"ejb{t0}")
            psv = ps[:].rearrange("p (j c) -> p j c", j=2)[:, 0:nt, 0:W]
            nc.scalar.activation(ej[:, 0:nt * W], psv, AF.Exp)
            for j in range(nt):
                t = t0 + j
                nc.vector.tensor_scalar(
                    ej[:, j * W:(j + 1) * W], ej[:, j * W:(j + 1) * W],
                    0.0, 0.0, ALU.add, ALU.add,
                    accum_out=se_b[t // H][:, t % H:t % H + 1])

        with tc.tile_pool(name="lgF", bufs=2, space="PSUM") as lgF:
            done = 0
            while done < B_SINGLES:
                b_half(lgF, done, 1)
                done += 1
            while done < T:
                nt = min(2, T - done)
                b_half(lgF, done, nt)
                done += nt
                # first half ready -> fold + ln early
                if done == H + 4:
                    nc.vector.tensor_tensor(se[0][:], se_a[0][:], se_b[0][:],
                                            ALU.add)
                    nc.scalar.activation(zbuf[0][:], se[0][:], AF.Ln,
                                         accum_out=zsum2[:, 0:1])

        # =============== finalize ======================================
        nc.vector.tensor_tensor(se[1][:], se_a[1][:], se_b[1][:], ALU.add)
        nc.scalar.activation(zbuf[1][:], se[1][:], AF.Ln,
                             accum_out=zsum2[:, 1:2])
        nc.vector.tensor_tensor(zsum[:], zsum2[:, 0:1], zsum2[:, 1:2], ALU.add)
        nc.gpsimd.partition_all_reduce(zred[:], zsum[:], P,
                                       bass_isa.ReduceOp.add)
        nc.vector.tensor_copy(outrow[:, 0:1], zred[0:1, :])
        nc.vector.tensor_tensor(outrow[:, 1:2], dotp[:, 0:1], dotp[:, 1:2],
                                ALU.add)
        nc.sync.dma_start(out_d.ap(), outrow[:])

        if dbg is not None:
            sgf = cpool.tile([P, 2 * W], F32, tag="sgf")
            nc.vector.tensor_copy(sgf[:], Sg[:].rearrange("p k c -> p (k c)"))
            nc.sync.dma_start(dbg["dbg_sg"].ap(), sgf[:])
            chf = cpool.tile([1, 16 * W], F32, tag="chf")
            nc.vector.tensor_copy(chf[:], ch[:])
            nc.sync.dma_start(dbg["dbg_ch"].ap(), chf[:])
            mof = cpool.tile([P, 2 * W], F32, tag="mof")
            nc.vector.tensor_copy(mof[:], mo8m[:].rearrange("p k c -> p (k c)"))
            nc.sync.dma_start(dbg["dbg_mo"].ap(), mof[:])
            sef = cpool.tile([P, T], F32, tag="sef")
            nc.vector.tensor_copy(sef[:, 0:H], se[0][:])
            nc.vector.tensor_copy(sef[:, H:T], se[1][:])
            nc.sync.dma_start(dbg["dbg_se"].ap(), sef[:])


def _dpair(mat_cd, dtype):
    """[C, D] -> [P, 2, C] with tile[ki, ko, c] = M[c, ko*128+ki]."""
    mt = mat_cd.T.reshape(2, P, mat_cd.shape[0])  # [ko, ki, c]
    return np.ascontiguousarray(mt.transpose(1, 0, 2).astype(dtype))


def _prep_inputs(feat, label, memory, source_memo):
    feat = np.asarray(feat, dtype=np.float32)
    label = np.asarray(label).astype(np.int64)
    memory = np.asarray(memory, dtype=np.float32)
    source_memo = np.asarray(source_memo, dtype=np.float32)

    nrm = np.maximum(np.sqrt((feat * feat).sum(axis=1, keepdims=True)),
                     np.float32(EPS))
    fn = (feat / nrm).astype(np.float32)

    order = np.argsort(label, kind="stable")
    fs_all = fn[order]
    ls_all = label[order]

    # per-core class bands (compile-time constants, shared SPMD program)
    los, spans = [], []
    for k in range(N_CORES):
        lk = int(ls_all[k * R])
        hk = int(ls_all[(k + 1) * R - 1])
        los.append(lk)
        spans.append(hk - lk + 1)
    cband = min(-(-max(spans) // 256) * 256, W)
    los = [min(lo, W - cband) for lo in los]

    mo8s = _dpair(source_memo, ml_dtypes.float8_e4m3fn).reshape(P, -1)
    memf = _dpair(memory, ml_dtypes.bfloat16).reshape(P, -1)

    in_maps = []
    for k in range(N_CORES):
        fs = fs_all[k * R:(k + 1) * R]
        ls = ls_all[k * R:(k + 1) * R]
        f4 = fs.reshape(TP, 2, P, D)               # [tp, ko, ki, d]
        fg8 = f4.transpose(2, 0, 1, 3).reshape(P, TP * 2 * D)
        fT8 = fs.T.reshape(2, P, R).transpose(1, 0, 2).reshape(P, 2 * R)
        rel = (ls - los[k]).reshape(TP, 2, P)       # [tp, ko, ki]
        oh4 = (rel[..., None] == np.arange(cband)[None, None, None, :])
        ohb = oh4.transpose(2, 0, 1, 3).reshape(P, TP * 2 * cband)
        in_maps.append({
            "fT8": np.ascontiguousarray(fT8.astype(ml_dtypes.float8_e4m3fn)),
            "fg8": np.ascontiguousarray(fg8.astype(ml_dtypes.float8_e4m3fn)),
            "ohb": np.ascontiguousarray(ohb.astype(ml_dtypes.float8_e4m3fn)),
            "mo8s": mo8s,
            "memf": memf,
        })
    return in_maps, cband, los


def _install_trace_hook():
    """The image's antenv lacks axon_hooks; recreate it from trn_agent_boot."""
    import sys, types
    import antenv
    if "antenv.axon_hooks" in sys.modules:
        return
    from trn_agent_boot.trn_boot import _ntff_profile_via_ctypes
    hook = _ntff_profile_via_ctypes("/opt/axon/libaxon_pjrt.so")
    m = types.ModuleType("antenv.axon_hooks")
    m.get_axon_ntff_profile_hook = lambda: hook
    sys.modules["antenv.axon_hooks"] = m
    antenv.axon_hooks = m
    import concourse.bass_utils as bu
    bu.upload_artifacts = lambda tmpdir: tmpdir


def _run(feat, label, memory, source_memo, trace=False, debug=False):
    if trace:
        _install_trace_hook()
    in_maps, cband, los = _prep_inputs(feat, label, memory, source_memo)
    key = (cband, tuple(los), debug)
    if key not in _CACHE:
        _CACHE[key] = _build(cband, los, debug)
    nc = _CACHE[key]
    res = run_bass_kernel_spmd(nc, in_maps, list(range(N_CORES)), trace=trace)
    zsum_total = sum(float(res.results[i]["out"][0, 0]) for i in range(N_CORES))
    dot = float(res.results[0]["out"][0, 1])
    loss = (zsum_total - dot) / N_TOTAL
    return np.asarray(loss, dtype=np.float32), res


def kernel(feat, label, memory, source_memo):
    loss, _ = _run(feat, label, memory, source_memo, trace=False)
    return loss
